# revision 1
# baseline (speedup 1.0000x reference)
"""Trainium2 Bass kernel for nn_DistMultMod, v7 (merged 1024-row gathers).

Decomposition (per core, BC=1024 triplets):
  - comp slots (masked last-writer nodes, ~200/core): dv = sum_k w*node_emb[neigh]
    via dma_gather of neighbor rows (bucketed by (parity tile T, 32768-row block))
    + PE matmuls with host-built scaled one-hot W (bf16).
  - score phase without DRAM scratch: per j-tile,
      psum_h[j] = sum_T Ah[T,j].T @ dv16[T]      (Ah[p,t] = a_slot one-hot, bf16)
      psum_r[j] = Rone[j].T @ rel_emb            (f32 one-hot over 16 rels)
      h = psum_h + g_h*old_h ; t = psum_t + g_t*old_t ; score = sum(h*t*rel)
    old head/tail rows fetched early with indirect DMAs (queue-0 Q7 pair).

v5 vs v3 changes:
  - num_swdge_queues=4: comp gathers spread across SWDGE queues 1-3 (+some on 0)
    so descriptor emission runs on all four Q7 core pairs concurrently
    (emission on one pair was 93% of the v3 critical path).
  - bf16 matmuls (W, gathered rows cast on DVE) instead of 2-pass fp32.
  - a/g blend coefficients computed on host; no device exp, no scratch
    round-trip, no slot/rel dma_gathers (PE one-hot matmuls instead).
  - idx16 pads are row 0 with w=0 (num_idxs_reg contract needs exact counts
    for -1 pads, which are input-dependent; pad-0 keeps the program static).
"""
import numpy as np
import ml_dtypes

BF16 = ml_dtypes.bfloat16

B = 8192
NCORES = 8
BC = B // NCORES        # 1024 triplets per core
D = 128
K = 64
N = 500000
RELS = 16
ND = 20000
NBLK = 16               # node_emb row blocks of 32768 (int16 index space)
BLK = 32768
NJ = BC // 128          # score tiles = 8
LAM = 0.7
SP = False              # single_packet=False for dma_gather

M_DEF = 256             # comp slots (NT=2 parity tiles x 128); last is dummy
GCAP_DEF = 4            # 128-row groups per bucket (512 rows capacity)

_CACHE = {}


def _queue_schedule(NOP):
    """Static GPSIMD emission schedule: greedy least-loaded interleave of
    16 indirect old-row gathers (queue 0 only) and NOP comp gathers
    (queues per COUNTS)."""
    COUNTS = {0: 3, 1: 5, 2: 4, 3: 4}
    assert sum(COUNTS.values()) == NOP
    qa = []             # queue of comp op i (in op order)
    rem = dict(COUNTS)
    cyc = [1, 2, 3, 0]
    ci = 0
    for _ in range(NOP):
        while rem[cyc[ci % 4]] == 0:
            ci += 1
        q = cyc[ci % 4]
        qa.append(q)
        rem[q] -= 1
        ci += 1
    items = {q: [] for q in range(4)}
    inds = []
    for j in range(NJ):
        inds.append(("ind", j, "h"))
        inds.append(("ind", j, "t"))
    for op, q in enumerate(qa):
        items[q].append(("comp", op, q))
    q0 = []
    for k, c in enumerate(items[0]):
        q0.append(c)
        q0.extend(inds[k * 5:(k + 1) * 5])
    q0.extend(inds[len(items[0]) * 5:])
    items[0] = q0
    DUR = {"ind": 1.2, "comp": 9.2}
    load = {q: 0.0 for q in range(4)}
    sched = []
    while any(items.values()):
        q = min((q for q in range(4) if items[q]), key=lambda q: load[q])
        it = items[q].pop(0)
        sched.append(it)
        load[q] += DUR[it[0]]
    return sched


def _prep_cores(head, rel, tailv, local_idx_map, sim_neighbors, sim_weights,
                degree_table, M, GCAP):
    NT = M // 128           # slot s -> (tile s%NT, partition s//NT)
    CAP = GCAP * 128
    NOP = NBLK * NT
    mask = (rel >= 2) & (rel <= 4)
    local_idx_map = np.asarray(local_idx_map)
    sim_neighbors = np.asarray(sim_neighbors)
    sim_weights = np.asarray(sim_weights)
    degree_table = np.asarray(degree_table)

    last_of = {}
    for b in range(B):
        last_of[int(head[b])] = b

    cores = []
    for c in range(NCORES):
        lo = c * BC
        slot_of = {}
        writers = []

        def slot_for(node):
            bw = last_of.get(int(node), -1)
            if bw < 0 or not mask[bw]:
                return M - 1
            s = slot_of.get(bw, -1)
            if s < 0:
                s = len(writers)
                slot_of[bw] = s
                writers.append(bw)
            return s

        slot_h = np.empty(BC, np.int32)
        slot_t = np.empty(BC, np.int32)
        for i in range(BC):
            slot_h[i] = slot_for(head[lo + i])
            slot_t[i] = slot_for(tailv[lo + i])
        m = len(writers)
        if m > M - 1:
            return "slots"

        a = np.zeros(M, np.float32)
        wl = np.array(writers, dtype=np.int64)
        if m:
            ls = local_idx_map[head[wl]]
            neigh_rows = sim_neighbors[ls].astype(np.int64)   # [m, K]
            w_rows = sim_weights[ls].astype(np.float32)       # [m, K]
            deg = degree_table[ls, rel[wl] - 2].astype(np.float32)
            a[:m] = LAM * np.exp(-LAM * deg) + 0.2
        else:
            neigh_rows = np.zeros((0, K), np.int64)
            w_rows = np.zeros((0, K), np.float32)

        # ---- bucket comp rows by (T = slot%NT, block) ----
        srows = np.repeat(np.arange(m), K)
        rnodes = neigh_rows.reshape(-1)
        wvals = w_rows.reshape(-1)
        Tpar = (srows % NT).astype(np.int64)
        blk = rnodes >> 15
        pcol = (srows // NT).astype(np.int64)
        off = (rnodes & (BLK - 1)).astype(np.int16)

        idx16 = np.zeros((NOP, CAP), np.int16)            # pad -> row 0, w=0
        wmat = np.zeros((NOP, GCAP, 128, 128), np.float32)
        order = np.lexsort((pcol, blk, Tpar))
        srt_T, srt_b = Tpar[order], blk[order]
        srt_off, srt_w, srt_p = off[order], wvals[order], pcol[order]
        for T in range(NT):
            for bk in range(NBLK):
                sel = np.flatnonzero((srt_T == T) & (srt_b == bk))
                nb = len(sel)
                if nb > CAP:
                    return "bucket"
                op = T * NBLK + bk
                idx16[op, :nb] = srt_off[sel]
                pos = np.arange(nb)
                wmat[op, pos // 128, pos % 128, srt_p[sel]] = srt_w[sel]

        # merge (T=0,bk) and (T=1,bk) buckets into one 1024-row gather op:
        # halves the SWDGE DMA instruction count (8-deep completion-sem
        # window is shared by all GPSIMD DMAs)
        NOPM = NBLK
        CAPM = 2 * CAP
        idx_m = np.concatenate([idx16[:NBLK], idx16[NBLK:]], axis=1)  # [16, 2CAP]
        wmat_m = np.concatenate([wmat[:NBLK], wmat[NBLK:]], axis=1)   # [16, 2G, 128, 128]
        idx16_w = np.zeros((NOPM, 128, CAPM // 16), np.int16)
        for op in range(NOPM):
            idx16_w[op] = np.tile(idx_m[op].reshape(CAPM // 16, 16).T, (8, 1))

        # ---- score-phase one-hot matrices ----
        i = np.arange(BC)
        j_, t_ = i // 128, i % 128
        Ah = np.zeros((128, NT * NJ * 128), np.float32)
        T_, p_ = slot_h % NT, slot_h // NT
        Ah[p_, (T_ * NJ + j_) * 128 + t_] = a[slot_h]
        At = np.zeros((128, NT * NJ * 128), np.float32)
        T_, p_ = slot_t % NT, slot_t // NT
        At[p_, (T_ * NJ + j_) * 128 + t_] = a[slot_t]
        gh = np.ascontiguousarray((1.0 - a[slot_h]).reshape(NJ, 128).T)
        gt = np.ascontiguousarray((1.0 - a[slot_t]).reshape(NJ, 128).T)
        rone = np.zeros((RELS, NJ * 128), np.float32)
        rone[rel[lo:lo + BC], i] = 1.0

        cores.append(dict(
            idx16=np.ascontiguousarray(
                idx16_w.transpose(1, 0, 2).reshape(128, NOP * (CAP // 16))),
            wmat=np.ascontiguousarray(
                wmat_m.reshape(NOP * GCAP, 128, 128)
                .transpose(1, 0, 2).reshape(128, NOP * GCAP * 128)
                ).astype(BF16),
            Ah=np.ascontiguousarray(Ah).astype(BF16),
            At=np.ascontiguousarray(At).astype(BF16),
            gh=gh.astype(np.float32),
            gt=gt.astype(np.float32),
            rone=np.ascontiguousarray(rone),
            headi=np.ascontiguousarray(
                head[lo:lo + BC].reshape(NJ, 128).T).astype(np.int32),
            taili=np.ascontiguousarray(
                tailv[lo:lo + BC].reshape(NJ, 128).T).astype(np.int32),
            n_slots=m,
        ))
    return cores


def _build_nc(M, GCAP):
    import concourse.bass as bass
    import concourse.bacc as bacc
    import concourse.mybir as mybir
    import concourse.tile as tile

    NT = M // 128
    CAP = GCAP * 128
    NOP = NBLK * NT
    f32 = mybir.dt.float32
    bf16 = mybir.dt.bfloat16
    i32 = mybir.dt.int32
    i16 = mybir.dt.int16
    Alu = mybir.AluOpType

    nc = bacc.Bacc("TRN2", target_bir_lowering=False, debug=False,
                   num_devices=NCORES, num_swdge_queues=4)

    node_emb = nc.dram_tensor("node_emb", [N, D], f32, kind="ExternalInput")
    rel_emb = nc.dram_tensor("rel_emb", [RELS, D], f32, kind="ExternalInput")
    idx16_t = nc.dram_tensor("idx16", [128, NOP * (CAP // 16)], i16,
                             kind="ExternalInput")
    wmat_t = nc.dram_tensor("wmat", [128, NOP * GCAP * 128], bf16,
                            kind="ExternalInput")
    Ah_t = nc.dram_tensor("Ah", [128, NT * NJ * 128], bf16, kind="ExternalInput")
    At_t = nc.dram_tensor("At", [128, NT * NJ * 128], bf16, kind="ExternalInput")
    gh_t = nc.dram_tensor("gh", [128, NJ], f32, kind="ExternalInput")
    gt_t = nc.dram_tensor("gt", [128, NJ], f32, kind="ExternalInput")
    rone_t = nc.dram_tensor("rone", [RELS, NJ * 128], f32, kind="ExternalInput")
    headi_t = nc.dram_tensor("headi", [128, NJ], i32, kind="ExternalInput")
    taili_t = nc.dram_tensor("taili", [128, NJ], i32, kind="ExternalInput")
    score_t = nc.dram_tensor("score", [128, NJ], f32, kind="ExternalOutput")

    NOPM = NBLK            # merged gather ops (both parity tiles per block)
    CAPM = 2 * CAP
    GC2 = 2 * GCAP
    sched = _queue_schedule(NOPM)
    comp_order = [it[1] for it in sched if it[0] == "comp"]
    first_op, last_op = comp_order[0], comp_order[-1]

    with tile.TileContext(nc) as tc:
        with tc.tile_pool(name="const", bufs=1) as constp, \
             tc.tile_pool(name="old", bufs=1) as oldp, \
             tc.tile_pool(name="gath", bufs=8) as gathp, \
             tc.tile_pool(name="g16", bufs=8) as g16p, \
             tc.tile_pool(name="wld", bufs=8) as wldp, \
             tc.tile_pool(name="cpsum", bufs=1, space="PSUM") as cpsump, \
             tc.tile_pool(name="spsum", bufs=2, space="PSUM") as spsump, \
             tc.tile_pool(name="work", bufs=4) as workp:

            idx_sb = constp.tile([128, NOP * (CAP // 16)], i16)
            nc.sync.dma_start(out=idx_sb[:], in_=idx16_t.ap())
            headi_sb = constp.tile([128, NJ], i32)
            nc.sync.dma_start(out=headi_sb[:], in_=headi_t.ap())
            taili_sb = constp.tile([128, NJ], i32)
            nc.sync.dma_start(out=taili_sb[:], in_=taili_t.ap())
            rone_sb = constp.tile([RELS, NJ * 128], f32)
            nc.sync.dma_start(out=rone_sb[:], in_=rone_t.ap())
            rel_sb = constp.tile([RELS, D], f32)
            nc.sync.dma_start(out=rel_sb[:], in_=rel_emb.ap())

            psts = [cpsump.tile([128, 128], f32, tag=f"ps{T}", name=f"ps{T}")
                    for T in range(NT)]
            old_h = [oldp.tile([128, D], f32, tag=f"oh{j}", name=f"oh{j}")
                     for j in range(NJ)]
            old_t = [oldp.tile([128, D], f32, tag=f"ot{j}", name=f"ot{j}")
                     for j in range(NJ)]
            dv16 = [constp.tile([128, D], bf16, tag=f"dv{T}", name=f"dv{T}")
                    for T in range(NT)]

            # ---- comp + old-row gathers, interleaved across SWDGE queues ----
            for it in sched:
                if it[0] == "ind":
                    _, j, hv = it
                    dst = old_h[j] if hv == "h" else old_t[j]
                    src = headi_sb if hv == "h" else taili_sb
                    nc.gpsimd.indirect_dma_start(
                        out=dst[:], out_offset=None, in_=node_emb.ap(),
                        in_offset=bass.IndirectOffsetOnAxis(
                            ap=src[:, j:j + 1], axis=0))
                    continue
                _, bk, q = it
                gt_ = gathp.tile([128, GC2 * D], f32, tag="g")
                nc.gpsimd.dma_gather(
                    out_ap=gt_[:].rearrange("p (b d) -> p b d", d=D),
                    in_ap=node_emb.ap()[bk * BLK:min((bk + 1) * BLK, N), :],
                    idxs_ap=idx_sb[:, bk * (CAPM // 16):(bk + 1) * (CAPM // 16)],
                    num_idxs=CAPM, num_idxs_reg=CAPM, elem_size=D,
                    single_packet=SP, queue_num=q)
                g16_ = g16p.tile([128, GC2 * D], bf16, tag="g16")
                nc.vector.tensor_copy(out=g16_[:], in_=gt_[:])
                wt_ = wldp.tile([128, GC2 * 128], bf16, tag="w")
                nc.sync.dma_start(
                    out=wt_[:],
                    in_=wmat_t.ap()[:, bk * GC2 * 128:(bk + 1) * GC2 * 128])
                for g in range(GC2):
                    T = 0 if g < GCAP else 1
                    nc.tensor.matmul(
                        out=psts[T][:],
                        lhsT=wt_[:, g * 128:(g + 1) * 128],
                        rhs=g16_[:, g * D:(g + 1) * D],
                        start=(bk == first_op and g % GCAP == 0),
                        stop=(bk == last_op and g % GCAP == GCAP - 1))
                if bk == last_op:
                    for T in range(NT):
                        nc.vector.tensor_copy(out=dv16[T][:], in_=psts[T][:])

            Ah_sb = constp.tile([128, NT * NJ * 128], bf16)
            nc.sync.dma_start(out=Ah_sb[:], in_=Ah_t.ap())
            At_sb = constp.tile([128, NT * NJ * 128], bf16)
            nc.sync.dma_start(out=At_sb[:], in_=At_t.ap())
            gh_sb = constp.tile([128, NJ], f32)
            nc.sync.dma_start(out=gh_sb[:], in_=gh_t.ap())
            gt_sb = constp.tile([128, NJ], f32)
            nc.sync.dma_start(out=gt_sb[:], in_=gt_t.ap())

            # ---- score phase ----
            score_sb = constp.tile([128, NJ], f32)
            for j in range(NJ):
                ph = spsump.tile([128, 128], f32, tag="ph")
                pt = spsump.tile([128, 128], f32, tag="pt")
                pr = spsump.tile([128, 128], f32, tag="pr")
                for T in range(NT):
                    nc.tensor.matmul(
                        out=ph[:],
                        lhsT=Ah_sb[:, (T * NJ + j) * 128:(T * NJ + j + 1) * 128],
                        rhs=dv16[T][:], start=(T == 0), stop=(T == NT - 1))
                for T in range(NT):
                    nc.tensor.matmul(
                        out=pt[:],
                        lhsT=At_sb[:, (T * NJ + j) * 128:(T * NJ + j + 1) * 128],
                        rhs=dv16[T][:], start=(T == 0), stop=(T == NT - 1))
                nc.tensor.matmul(
                    out=pr[:], lhsT=rone_sb[:, j * 128:(j + 1) * 128],
                    rhs=rel_sb[:], start=True, stop=True)

                t2 = workp.tile([128, D], f32, tag="t2")
                nc.vector.tensor_scalar(
                    out=t2[:], in0=old_h[j][:], scalar1=gh_sb[:, j:j + 1],
                    scalar2=None, op0=Alu.mult)
                hv = workp.tile([128, D], f32, tag="hv")
                nc.vector.tensor_tensor(out=hv[:], in0=ph[:], in1=t2[:],
                                        op=Alu.add)
                t4 = workp.tile([128, D], f32, tag="t4")
                nc.vector.tensor_scalar(
                    out=t4[:], in0=old_t[j][:], scalar1=gt_sb[:, j:j + 1],
                    scalar2=None, op0=Alu.mult)
                tv = workp.tile([128, D], f32, tag="tv")
                nc.vector.tensor_tensor(out=tv[:], in0=pt[:], in1=t4[:],
                                        op=Alu.add)
                p1 = workp.tile([128, D], f32, tag="p1")
                nc.vector.tensor_tensor(out=p1[:], in0=hv[:], in1=tv[:],
                                        op=Alu.mult)
                p2 = workp.tile([128, D], f32, tag="p2")
                nc.vector.tensor_tensor(out=p2[:], in0=p1[:], in1=pr[:],
                                        op=Alu.mult)
                nc.vector.reduce_sum(out=score_sb[:, j:j + 1], in_=p2[:],
                                     axis=mybir.AxisListType.X)
            nc.sync.dma_start(out=score_t.ap(), in_=score_sb[:])

    nc.compile()
    return nc


def _get_nc(M=M_DEF, GCAP=GCAP_DEF):
    key = (M, GCAP)
    if key not in _CACHE:
        _CACHE[key] = _build_nc(M, GCAP)
    return _CACHE[key]


def kernel(head_index, rel_type, tail_index, node_emb, rel_emb,
           local_idx_map, sim_neighbors, sim_weights, degree_table):
    from concourse.bass_utils import run_bass_kernel_spmd

    head = np.asarray(head_index).astype(np.int64)
    rel = np.asarray(rel_type).astype(np.int64)
    tailv = np.asarray(tail_index).astype(np.int64)
    node_emb = np.ascontiguousarray(np.asarray(node_emb, dtype=np.float32))
    rel_emb = np.ascontiguousarray(np.asarray(rel_emb, dtype=np.float32))

    M, GCAP = M_DEF, GCAP_DEF
    while True:
        cores = _prep_cores(head, rel, tailv, local_idx_map, sim_neighbors,
                            sim_weights, degree_table, M, GCAP)
        if isinstance(cores, list):
            break
        if cores == "slots":
            M *= 2
        else:
            GCAP += 2

    nc = _get_nc(M, GCAP)
    in_maps = []
    for c in range(NCORES):
        cc = cores[c]
        in_maps.append({
            "node_emb": node_emb, "rel_emb": rel_emb,
            "idx16": cc["idx16"], "wmat": cc["wmat"],
            "Ah": cc["Ah"], "At": cc["At"],
            "gh": cc["gh"], "gt": cc["gt"], "rone": cc["rone"],
            "headi": cc["headi"], "taili": cc["taili"],
        })

    _CACHE["last_in_maps"] = in_maps
    res = run_bass_kernel_spmd(nc, in_maps, core_ids=list(range(NCORES)))
    _CACHE["last_result"] = res
    _CACHE["last_nc"] = nc

    out = np.empty(B, np.float32)
    for c in range(NCORES):
        out[c * BC:(c + 1) * BC] = res.results[c]["score"].T.reshape(-1)
    return out

